# revision 19
# baseline (speedup 1.0000x reference)
"""NT-Xent loss kernel for Trainium2, 8 NeuronCores.

Strategy (row-sharded similarity matrix):
  - Each core receives the full feature matrix cyclically rolled by c*1024
    rows, so every core runs the identical program: its 1024 rows are
    rolled-rows [0, 1024), its positive columns are [4096, 5120).
  - Column groups outermost (g=0..3), row tiles m=0..7, 1024-col halves
    innermost; group g+1's normalize/transpose overlaps group g's exp
    stream.  Four rotating [128,1024] PSUM slots hide the per-slot
    matmul->exp handoff latency that a 2x2048 layout exposes.
  - Exp extraction balances ScalarE and the DVE: ACT chunks run exact
    exp (activation, accum_out = row sums); DVE_CHUNKS run an int16
    Schraudolph -- int16(sim*A16 + B16) into a NATIVE int16 tile is a
    2x-mode DVE op (2-byte packed output); its bits bitcast to bf16 are
    a ~2%-accurate exp(10*sim) whose row-sum error washes out.  The
    row-sum is a 1x tensor_reduce (no DVE fast mode reaches f32
    outputs, measured).
  - Normalization: squares on GPSIMD (f32 -> bf16), segmented reduces
    on DVE in bf16 (2-byte in AND out -> 2x mode) then a tiny cast to
    f32; rsqrt via the magic-constant bit trick (DVE seed) + 2
    tensor-tensor-only Newton steps on GPSIMD; zbg = xg * rno as
    broadcast-AP tensor_tensor ops on GPSIMD (f32 -> bf16).
  - PE transposes write into a just-consumed psum tile (explicit WAR dep
    on its exp) so no PSUM slot is ever held across chunks; DVE copies
    psum -> zbT (2x mode: bf16 out).  Dummy matmuls keep the PE HAM
    clock gate warm (cold PE runs 1.2 GHz, warm 2.4 GHz).
  - Diagonal self-sim is exp(10) exactly (z normalized) -> subtract a
    constant instead of extracting it.  Positives are computed directly
    as a bf16 dot product of zbg (rows block) with zbg (positive block).
  - loss_row = ln(rowsum - e^10) - 10*pos; per-core [128, 8] tile is
    DMA'd out; the host sums partials and divides by N.
"""

import os

import numpy as np

N = 8192
D = 128
NCORES = 8
RPC = N // NCORES          # rows per core = 1024
G = 4                      # column groups
GCOLS = N // G             # 2048 columns per group
RT = RPC // 128            # row tiles per core = 8
ESC = 10.0                 # 1 / temperature
E10 = float(np.exp(10.0))  # diagonal exp value (z normalized -> sim_ii = 10)

# int16 Schraudolph constants: bf16_bits(exp(10*s)) ~ int16(s*A16 + B16).
A16 = 10.0 * (2.0 ** 7) / float(np.log(2.0))    # 1846.63
B16 = float(127 * 2 ** 7 - 7.25)                # calibrated for zero mean err

# (g, m, h) half-chunks computed on the DVE instead of ScalarE.  g=0 must
# stay on ACT (diagonal runs through exact exp so e^10 cancels exactly).
DVE_CHUNKS = (
    {(g, m, h) for g in (1, 2, 3) for m in (1, 4, 6) for h in (0, 1)}
    | {(3, 2, 0), (3, 2, 1)}
)

_CACHE = {}
LAST_RESULTS = None


def _patch_act_tables():
    """Force Exp/Ln onto the combined natural_log_exp_and_others table set.

    The greedy table-load pass otherwise alternates between exp-only and
    ln-only sets (one ~1.3us table load per switch).  Stripping Exp/Ln
    from the competing sets leaves exactly one set that can serve them,
    so a single load covers the whole kernel.
    """
    if _CACHE.get("act_patched"):
        return
    import functools

    import concourse.bacc as bacc_mod
    import concourse.bass_interp as interp_mod
    import concourse.hw_specs as hw_specs
    import concourse.mybir as mybir

    AF = mybir.ActivationFunctionType
    orig = hw_specs.get_activation_tables

    @functools.cache
    def patched(arch):
        out = {}
        for name, funcs in orig(arch).items():
            if name != "natural_log_exp_and_others":
                funcs = funcs - {AF.Exp, AF.Ln}
            out[name] = funcs
        return out

    hw_specs.get_activation_tables = patched
    bacc_mod.get_activation_tables = patched
    interp_mod.get_activation_tables = patched
    _CACHE["act_patched"] = True


def _build():
    import concourse.mybir as mybir
    import concourse.tile as tile
    from concourse import bacc
    from bass_rust import add_dep_helper

    _patch_act_tables()

    f32 = mybir.dt.float32
    bf16 = mybir.dt.bfloat16
    i16 = mybir.dt.int16
    i32 = mybir.dt.int32
    AX = mybir.AxisListType
    OP = mybir.AluOpType
    AF = mybir.ActivationFunctionType

    nc = bacc.Bacc(
        "TRN2",
        target_bir_lowering=False,
        debug=False,
        enable_asserts=False,
        num_devices=NCORES,
    )
    x = nc.dram_tensor("x", [N, D], f32, kind="ExternalInput").ap()
    ident_in = nc.dram_tensor("ident", [128, 128], f32, kind="ExternalInput").ap()
    out = nc.dram_tensor("loss_parts", [128, RT], f32, kind="ExternalOutput").ap()

    with tile.TileContext(nc) as tc:
        with (
            tc.tile_pool(name="const", bufs=1) as constp,
            tc.tile_pool(name="big", bufs=1) as bigp,
            tc.tile_pool(name="small", bufs=2) as smallp,
            tc.tile_pool(name="psum", bufs=4, space="PSUM") as psump,
        ):
            ident = constp.tile([128, 128], bf16, tag="ident")
            identf = constp.tile([128, 128], f32, tag="identf")
            nc.scalar.dma_start(out=identf[:], in_=ident_in)
            nc.vector.tensor_copy(ident[:], identf[:])

            # Touch Ln+Exp so the ACT table load starts early.
            warm = constp.tile([128, 1], f32, tag="warm")
            nc.vector.memset(warm[:], 1.0)
            nc.scalar.activation(warm[:], warm[:], AF.Ln)
            nc.scalar.activation(warm[:], warm[:], AF.Exp)

            eps2 = constp.tile([128, 1], f32, tag="eps2")
            nc.vector.memset(eps2[:], 1e-16)
            c15 = constp.tile([128, 16], f32, tag="c15")
            nc.vector.memset(c15[:], 1.5)

            # Dedicated (non-rotating) tiles: lifetimes are simple and SBUF
            # is plentiful, so avoid pool-recycling hazards entirely.
            xg = [bigp.tile([128, GCOLS], f32, tag=f"xg{g}", name=f"xg{g}") for g in range(G)]
            zbg = [bigp.tile([128, GCOLS], bf16, tag=f"zbg{g}", name=f"zbg{g}") for g in range(G)]
            zbT = [bigp.tile([128, GCOLS], bf16, tag=f"zbT{g}", name=f"zbT{g}") for g in range(G)]
            sqs = [bigp.tile([128, GCOLS], bf16, tag=f"sqs{k}", name=f"sqs{k}") for k in range(2)]
            nsb = [bigp.tile([128, 16], bf16, tag=f"nsb{g}", name=f"nsb{g}") for g in range(G)]
            nsq = [bigp.tile([128, 16], f32, tag=f"nsq{g}", name=f"nsq{g}") for g in range(G)]
            rno = [bigp.tile([128, 16], f32, tag=f"rno{g}", name=f"rno{g}") for g in range(G)]
            # exp destinations (ACT chunks) and native-int16 Schraudolph
            # tiles (DVE chunks; 2-byte packed output -> 2x DVE mode)
            et = [bigp.tile([128, 1024], bf16, tag=f"et{k}", name=f"et{k}") for k in range(2)]
            ei = [bigp.tile([128, 1024], i16, tag=f"ei{k}", name=f"ei{k}") for k in range(2)]

            racc = constp.tile([128, 2 * G * RT + 4], f32, tag="racc")
            pos = constp.tile([128, RT], f32, tag="pos")
            nc.vector.memset(racc[:], 0.0)

            def load_group(g):
                """DMA 512-row chunks of group g."""
                for q in range(4):
                    src = x[g * GCOLS + q * 512 : g * GCOLS + (q + 1) * 512, :]
                    src = src.rearrange("(p s) d -> p s d", p=128)
                    dst = xg[g][:, q * 512 : (q + 1) * 512].rearrange(
                        "p (s d) -> p s d", s=4
                    )
                    eng = nc.sync if q % 2 == 0 else nc.scalar
                    eng.dma_start(out=dst, in_=src)

            def norm_chunk(g, q):
                """squares (GPSIMD, f32->bf16) + 2x bf16 segmented reduce."""
                sl = slice(q * 512, (q + 1) * 512)
                sq = sqs[g % 2]
                nc.gpsimd.tensor_mul(sq[:, sl], xg[g][:, sl], xg[g][:, sl])
                with nc.allow_low_precision(
                    reason="norm sums accumulate f32 internally; bf16 store "
                           "costs ~0.2% on rno, invisible after the row-sum"
                ):
                    nc.vector.tensor_reduce(
                        nsb[g][:, q * 4 : (q + 1) * 4],
                        sq[:, sl].rearrange("p (s d) -> p s d", s=4),
                        axis=AX.X, op=OP.add,
                    )

            def norm_cast(g):
                """nsq = f32(nsb) -- one tiny DVE cast per group."""
                return nc.vector.tensor_copy(nsq[g][:], nsb[g][:])

            def norm_act(g, dep=None):
                """head variant: rno via ACT ln/exp (ACT is idle early)."""
                lnv = smallp.tile([128, 16], f32, tag="lnv")
                ln_i = nc.scalar.activation(lnv[:], nsq[g][:],
                                            AF.Ln, bias=eps2[:, 0:1])
                if dep is not None:
                    add_dep_helper(ln_i.ins, dep.ins, sync=True,
                                   reason="ln waits norm cast")
                nc.scalar.activation(rno[g][:], lnv[:], AF.Exp, scale=-0.5)

            def rsqrt_group(g):
                """rno = 1/sqrt(nsq): DVE bit-trick seed + 2 TT-only Newton
                steps on GPSIMD (keeps steady-state work off ACT and DVE)."""
                ii = smallp.tile([128, 16], i32, tag="ii")
                nc.vector.tensor_scalar(
                    out=ii[:], in0=nsq[g][:].bitcast(i32),
                    scalar1=1, scalar2=None, op0=OP.logical_shift_right,
                )
                magic = smallp.tile([128, 16], i32, tag="magic")
                nc.vector.memset(magic[:], 0x5F3759DF)
                y0i = smallp.tile([128, 16], i32, tag="y0i")
                nc.vector.tensor_sub(y0i[:], magic[:], ii[:])
                nsqh = smallp.tile([128, 16], f32, tag="nsqh")
                nc.vector.tensor_scalar_mul(nsqh[:], nsq[g][:], 0.5)
                y0 = y0i[:].bitcast(f32)
                t = smallp.tile([128, 16], f32, tag="t")
                for _ in range(2):
                    nc.gpsimd.tensor_mul(t[:], y0, y0)
                    nc.gpsimd.tensor_mul(t[:], t[:], nsqh[:])
                    nc.gpsimd.tensor_sub(t[:], c15[:], t[:])
                    nc.gpsimd.tensor_mul(rno[g][:], y0, t[:])
                    y0 = rno[g][:]

            def scale_chunk(g, q, dve=False):
                """zbg = xg * rno (broadcast TT per 512-col chunk, f32->bf16).

                GPSIMD in steady state; DVE for the latency-critical head.
                """
                eng = nc.vector if dve else nc.gpsimd
                sl = slice(q * 512, (q + 1) * 512)
                b = rno[g][:, 4 * q : 4 * q + 4].broadcast_to([128, 4, 128])
                eng.tensor_mul(
                    zbg[g][:, sl].rearrange("p (s d) -> p s d", s=4),
                    xg[g][:, sl].rearrange("p (s d) -> p s d", s=4),
                    b,
                )

            def transpose_chunk(g, q, ptr, dep=None):
                """PE-transpose 512 cols of zbg into psum, DVE-copy to zbT.

                ptr is a [128, 512] bf16 view carved out of a pt-pool tile
                that has just been consumed, so no extra PSUM slot is held.
                """
                for j in range(4):
                    s = 4 * q + j
                    tr = nc.tensor.transpose(
                        ptr[:, j * 128 : (j + 1) * 128],
                        zbg[g][:, s * 128 : (s + 1) * 128],
                        ident[:],
                    )
                    if dep is not None and j == 0:
                        add_dep_helper(tr.ins, dep.ins, sync=True,
                                       reason="transpose WAR on psum consumer")
                nc.vector.tensor_copy(
                    zbT[g][:, q * 512 : (q + 1) * 512], ptr[:]
                )

            last_exp = [None]
            last_dve = [None]

            def mm_chunk(g, m, h):
                """One 1024-col half-chunk: 2 matmuls + exp (ACT or DVE)."""
                pt = psump.tile([128, 1024], f32, tag="pt")
                lhs = zbT[0][:, m * 128 : (m + 1) * 128]
                for k in range(2):
                    c = h * 1024 + k * 512
                    nc.tensor.matmul(
                        pt[:, k * 512 : (k + 1) * 512],
                        lhs,
                        zbT[g][:, c : c + 512],
                    )
                col = (g * RT + m) * 2 + h
                if (g, m, h) in DVE_CHUNKS:
                    # 2x Schraudolph: int16(s*A16+B16) = bf16 bits of exp(10s)
                    cons = nc.vector.tensor_scalar(
                        out=ei[h][:],
                        in0=pt[:],
                        scalar1=A16,
                        scalar2=B16,
                        op0=OP.mult,
                        op1=OP.add,
                    )
                    red = nc.vector.tensor_reduce(
                        racc[:, col : col + 1],
                        ei[h][:].bitcast(bf16),
                        axis=AX.X, op=OP.add,
                    )
                    last_dve[0] = red
                else:
                    cons = nc.scalar.activation(
                        et[h][:], pt[:], AF.Exp, scale=ESC,
                        accum_out=racc[:, col : col + 1],
                    )
                    last_exp[0] = cons
                return pt, cons

            # ---- head: group 0.  All norms first (GPSIMD squares pipeline,
            # DVE segreds), one ACT ln/exp for all 16 blocks, then per-q
            # scale/transpose/mm/exp with no cross-q serial chain.  Pool
            # tiles host the m=0 strips, transpose scratch, and dummy-matmul
            # warm strip.  Dummy matmuls keep the PE HAM clock gate warm.
            dumm = bigp.tile([128, 128], bf16, tag="dumm")
            nc.vector.memset(dumm[:], 0.0)
            pth = [psump.tile([128, 1024], f32, tag="pt", name=f"pth{q}")
                   for q in range(4)]

            def pe_warm(t, n):
                for _ in range(n):
                    nc.tensor.matmul(t[:, 512:640], ident[:], dumm[:])

            pe_warm(pth[0], 40)
            load_group(0)
            for q in range(4):
                norm_chunk(0, q)
            norm_act(0, dep=norm_cast(0))
            for q in range(4):
                ph = pth[q]
                scale_chunk(0, q, dve=True)
                trv = ph.bitcast(bf16)[:, 1536:2048]
                transpose_chunk(0, q, trv)
                nc.tensor.matmul(
                    ph[:, 0:512],
                    zbT[0][:, 0:128],
                    zbT[0][:, q * 512 : (q + 1) * 512],
                )
                nc.scalar.activation(
                    et[0][:, 0:512],
                    ph[:, 0:512],
                    AF.Exp, scale=ESC,
                    accum_out=racc[:, 2 * G * RT + q : 2 * G * RT + q + 1],
                )
                if q < 3:
                    pe_warm(pth[q + 1], 8)
            load_group(1)
            # ---- main stream: g outer, m inner, 1024-col halves.  Four
            # rotating PSUM slots hide the per-slot handoff latency.  Group
            # g+1's transposes run as bursts of 8 into just-consumed psum
            # tiles, so no extra PSUM slot is ever held.
            for g in range(G):
                if g == 0:
                    mh_order = [(m, h) for m in range(1, RT) for h in range(2)]
                else:
                    mh_order = [(m, h) for m in range(RT) for h in range(2)]
                for i, (m, h) in enumerate(mh_order):
                    pt, cons = mm_chunk(g, m, h)
                    if g + 1 < G:
                        bpos = {4: 0, 8: 2} if g == 0 else {6: 0, 10: 2}
                        burst = bpos.get(i)
                        if burst is not None:
                            trv = pt.bitcast(bf16)
                            transpose_chunk(g + 1, burst, trv[:, 0:512], dep=cons)
                            transpose_chunk(g + 1, burst + 1, trv[:, 512:1024],
                                            dep=cons)
                        if i in (0, 1):
                            norm_chunk(g + 1, 2 * i)
                            norm_chunk(g + 1, 2 * i + 1)
                        elif i == 2:
                            norm_cast(g + 1)
                            rsqrt_group(g + 1)
                        elif i in (3, 4):
                            scale_chunk(g + 1, 2 * (i - 3))
                            scale_chunk(g + 1, 2 * (i - 3) + 1)
                        elif i == 12 and g + 2 < G:
                            load_group(g + 2)
                if g == 2:
                    # positives: pos[p, s] = sum_d zbg0[p,s,d] * zbg2[p,s,d]
                    pz = bigp.tile([128, RPC], bf16, tag="pz")
                    nc.gpsimd.tensor_mul(pz[:], zbg[0][:, 0:RPC], zbg[2][:, 0:RPC])
                    nc.vector.tensor_reduce(
                        pos[:],
                        pz[:].rearrange("p (s d) -> p s d", s=RT),
                        axis=AX.X, op=OP.add,
                    )

            # ---- epilogue: loss = ln(rowsum - e^10) - 10*pos ----
            tot32 = smallp.tile([128, G * RT], f32, tag="tot32")
            t32_i = nc.vector.tensor_reduce(
                tot32[:],
                racc[:, 0 : 2 * G * RT].rearrange("p (gm h) -> p gm h", h=2),
                axis=AX.X, op=OP.add,
            )
            if last_exp[0] is not None:
                add_dep_helper(t32_i.ins, last_exp[0].ins, sync=True,
                               reason="epilogue waits last ACT accum_out")
            if last_dve[0] is not None:
                add_dep_helper(t32_i.ins, last_dve[0].ins, sync=True,
                               reason="epilogue waits last DVE reduce")
            tot = smallp.tile([128, RT], f32, tag="tot")
            nc.vector.tensor_reduce(
                tot[:],
                tot32[:].rearrange("p (g m) -> p m g", m=RT),
                axis=AX.X, op=OP.add,
            )
            th = smallp.tile([128, 1], f32, tag="th")
            nc.vector.tensor_reduce(
                th[:], racc[:, 2 * G * RT : 2 * G * RT + 4], axis=AX.X, op=OP.add
            )
            # fold the head sub-chunk sums into m=0
            nc.vector.tensor_add(tot[:, 0:1], tot[:, 0:1], th[:])
            ndall = smallp.tile([128, RT], f32, tag="ndall")
            nc.vector.tensor_scalar_add(ndall[:], tot[:], -E10)
            lnd = smallp.tile([128, RT], f32, tag="lnd")
            nc.scalar.activation(lnd[:], ndall[:], AF.Ln)
            lt = smallp.tile([128, RT], f32, tag="lt")
            nc.vector.scalar_tensor_tensor(
                out=lt[:], in0=pos[:], scalar=-ESC, in1=lnd[:],
                op0=OP.mult, op1=OP.add,
            )
            nc.sync.dma_start(out=out, in_=lt[:])

    nc.compile()
    return nc


def _get_nc():
    if "nc" not in _CACHE:
        _CACHE["nc"] = _build()
    return _CACHE["nc"]


def kernel(stacked_batch: np.ndarray) -> np.ndarray:
    global LAST_RESULTS
    from concourse.bass_utils import run_bass_kernel_spmd

    nc = _get_nc()
    xf = np.ascontiguousarray(np.asarray(stacked_batch, dtype=np.float32))
    assert xf.shape == (N, D)

    ident = np.eye(128, dtype=np.float32)
    in_maps = [
        {"x": np.ascontiguousarray(np.roll(xf, -c * RPC, axis=0)), "ident": ident}
        for c in range(NCORES)
    ]
    res = run_bass_kernel_spmd(
        nc,
        in_maps,
        core_ids=list(range(NCORES)),
        trace=bool(os.environ.get("BASS_TRACE")),
    )
    LAST_RESULTS = res
    total = 0.0
    for c in range(NCORES):
        total += float(np.asarray(res.results[c]["loss_parts"], dtype=np.float64).sum())
    return np.float32(total / N)


# revision 22
# speedup vs baseline: 1.0642x; 1.0642x over previous
"""NT-Xent loss kernel for Trainium2, 8 NeuronCores.

Strategy (row-sharded similarity matrix):
  - Each core receives the full feature matrix cyclically rolled by c*1024
    rows, so every core runs the identical program: its 1024 rows are
    rolled-rows [0, 1024), its positive columns are [4096, 5120).
  - Column groups outermost (g=0..3), row tiles m=0..7; each (g, m) chunk
    is a full [128, 2048] PSUM tile (4 matmuls) so the per-instruction
    overhead of the exp stream is amortized over 2048 elements.  Two
    rotating [128, 2048] PSUM slots ping-pong between PE fill and
    ACT/DVE drain.
  - Exp extraction is split across engines to balance their busy time:
    most chunks run exact exp on ScalarE (activation, accum_out = row
    sums); DVE_CHUNKS run an int16 Schraudolph on the DVE --
    int16(sim*A16 + B16) bitcast to bf16 is a ~2%-accurate exp(10*sim)
    -- followed by a 4x-mode tensor_scalar (scalar1=1, accum_out) that
    reduces the bf16 tile at 0.26 ns/elem.
  - Normalization: GPSIMD casts xg -> bf16, DVE squares (bf16 2x) and
    rsqrt via the magic-constant bit trick + 2 Newton steps; segmented
    norm reduces on GPSIMD (DVE for the head group); zbg = xbg * rno as
    per-block 4x-mode DVE tensor_scalars.
  - PE transposes write into a just-consumed psum slot (explicit WAR dep
    on its consumer); GPSIMD copies psum -> zbT.  Dummy matmuls up front
    keep the PE HAM clock gate warm (cold PE runs 1.2 GHz, warm 2.4).
  - Diagonal self-sim is exp(10) exactly (z normalized) -> subtract a
    constant instead of extracting it.  Positives are computed directly
    as a bf16 dot product of zbg (rows block) with zbg (positive block).
  - loss_row = ln(rowsum - e^10) - 10*pos; per-core [128, 8] tile is
    DMA'd out; the host sums partials and divides by N.
"""

import os

import numpy as np

N = 8192
D = 128
NCORES = 8
RPC = N // NCORES          # rows per core = 1024
G = 4                      # column groups
GCOLS = N // G             # 2048 columns per group
RT = RPC // 128            # row tiles per core = 8
ESC = 10.0                 # 1 / temperature
E10 = float(np.exp(10.0))  # diagonal exp value (z normalized -> sim_ii = 10)

# int16 Schraudolph constants: bf16_bits(exp(10*s)) ~ int16(s*A16 + B16).
A16 = 10.0 * (2.0 ** 7) / float(np.log(2.0))    # 1846.63
B16 = float(127 * 2 ** 7 - 7.25)                # calibrated for zero mean err

# (g, m) chunks computed on the DVE instead of ScalarE.  g=0 stays on ACT
# (diagonal runs through exact exp so the e^10 subtraction cancels).
DVE_CHUNKS = {(1, 1), (1, 4),
              (2, 1), (2, 4),
              (3, 1), (3, 4), (3, 6)}

_CACHE = {}
LAST_RESULTS = None


def _patch_act_tables():
    """Force Exp/Ln onto the combined natural_log_exp_and_others table set.

    The greedy table-load pass otherwise alternates between exp-only and
    ln-only sets (one table load per switch).  Stripping Exp/Ln from the
    competing sets leaves exactly one set that can serve them, so a
    single load covers the whole kernel.
    """
    if _CACHE.get("act_patched"):
        return
    import functools

    import concourse.bacc as bacc_mod
    import concourse.bass_interp as interp_mod
    import concourse.hw_specs as hw_specs
    import concourse.mybir as mybir

    AF = mybir.ActivationFunctionType
    orig = hw_specs.get_activation_tables

    @functools.cache
    def patched(arch):
        out = {}
        for name, funcs in orig(arch).items():
            if name != "natural_log_exp_and_others":
                funcs = funcs - {AF.Exp, AF.Ln}
            out[name] = funcs
        return out

    hw_specs.get_activation_tables = patched
    bacc_mod.get_activation_tables = patched
    interp_mod.get_activation_tables = patched
    _CACHE["act_patched"] = True


def _build():
    import concourse.mybir as mybir
    import concourse.tile as tile
    from concourse import bacc
    from bass_rust import add_dep_helper

    _patch_act_tables()

    f32 = mybir.dt.float32
    bf16 = mybir.dt.bfloat16
    i16 = mybir.dt.int16
    i32 = mybir.dt.int32
    AX = mybir.AxisListType
    OP = mybir.AluOpType
    AF = mybir.ActivationFunctionType

    nc = bacc.Bacc(
        "TRN2",
        target_bir_lowering=False,
        debug=False,
        enable_asserts=False,
        num_devices=NCORES,
    )
    x = nc.dram_tensor("x", [N, D], f32, kind="ExternalInput").ap()
    ident_in = nc.dram_tensor("ident", [128, 128], f32, kind="ExternalInput").ap()
    out = nc.dram_tensor("loss_parts", [128, RT], f32, kind="ExternalOutput").ap()

    with tile.TileContext(nc) as tc:
        with (
            tc.tile_pool(name="const", bufs=1) as constp,
            tc.tile_pool(name="big", bufs=1) as bigp,
            tc.tile_pool(name="small", bufs=2) as smallp,
            tc.tile_pool(name="psum", bufs=2, space="PSUM") as psump,
        ):
            ident = constp.tile([128, 128], bf16, tag="ident")
            identf = constp.tile([128, 128], f32, tag="identf")
            nc.scalar.dma_start(out=identf[:], in_=ident_in)
            nc.vector.tensor_copy(ident[:], identf[:])

            # Touch Ln+Exp so the ACT table load starts early.
            warm = constp.tile([128, 1], f32, tag="warm")
            nc.vector.memset(warm[:], 1.0)
            nc.scalar.activation(warm[:], warm[:], AF.Ln)
            nc.scalar.activation(warm[:], warm[:], AF.Exp)

            eps2 = constp.tile([128, 1], f32, tag="eps2")
            nc.vector.memset(eps2[:], 1e-16)
            c15 = constp.tile([128, 16], f32, tag="c15")
            nc.vector.memset(c15[:], 1.5)

            # Dedicated (non-rotating) tiles: lifetimes are simple and SBUF
            # is plentiful, so avoid pool-recycling hazards entirely.
            xg = [bigp.tile([128, GCOLS], f32, tag=f"xg{g}", name=f"xg{g}") for g in range(G)]
            zbg = [bigp.tile([128, GCOLS], bf16, tag=f"zbg{g}", name=f"zbg{g}") for g in range(G)]
            zbT = [bigp.tile([128, GCOLS], bf16, tag=f"zbT{g}", name=f"zbT{g}") for g in range(G)]
            nsq = [bigp.tile([128, 16], f32, tag=f"nsq{g}", name=f"nsq{g}") for g in range(G)]
            rno = [bigp.tile([128, 16], f32, tag=f"rno{g}", name=f"rno{g}") for g in range(G)]
            sqs = [bigp.tile([128, GCOLS], bf16, tag=f"sqs{k}", name=f"sqs{k}") for k in range(2)]
            sdum = bigp.tile([128, 128], bf16, tag="sdum", name="sdum")
            # exp destinations (ACT chunks), Schraudolph ints + reduce
            # scratch (DVE chunks)
            et = [bigp.tile([128, GCOLS], bf16, tag=f"et{k}", name=f"et{k}") for k in range(2)]
            ei = [bigp.tile([128, GCOLS], bf16, tag=f"ei{k}", name=f"ei{k}") for k in range(2)]
            eb = bigp.tile([128, GCOLS], bf16, tag="eb", name="eb")

            racc = constp.tile([128, G * RT + 4], f32, tag="racc")
            pos = constp.tile([128, RT], f32, tag="pos")
            nc.vector.memset(racc[:], 0.0)  # col (0,0) is covered by head cols

            def load_group(g):
                """DMA 512-row chunks of group g."""
                for q in range(4):
                    src = x[g * GCOLS + q * 512 : g * GCOLS + (q + 1) * 512, :]
                    src = src.rearrange("(p s) d -> p s d", p=128)
                    dst = xg[g][:, q * 512 : (q + 1) * 512].rearrange(
                        "p (s d) -> p s d", s=4
                    )
                    eng = nc.sync if q % 2 == 0 else nc.scalar
                    eng.dma_start(out=dst, in_=src)

            def norm_chunk(g, q):
                """nsq[:, s] = sum_d xg[:, s-block]^2 -- one-pass TTR on DVE."""
                for j in range(4):
                    s = 4 * q + j
                    blk = xg[g][:, s * 128 : (s + 1) * 128]
                    nc.vector.tensor_tensor_reduce(
                        out=sqd[:], in0=blk, in1=blk,
                        scale=1.0, scalar=0.0, op0=OP.mult, op1=OP.add,
                        accum_out=nsq[g][:, s : s + 1],
                    )

            def rsqrt_group(g, dep=None):
                """rno = 1/sqrt(nsq) on GPSIMD: magic-constant seed + 2
                Newton steps (~1e-5 rel err; the eps clamp is irrelevant for
                randn data).  Keeps the rno ops off ScalarE and the DVE."""
                eng = nc.gpsimd
                ii = smallp.tile([128, 16], i32, tag="ii")
                sh_i = nc.vector.tensor_scalar(
                    out=ii[:], in0=nsq[g][:].bitcast(i32),
                    scalar1=1, scalar2=None, op0=OP.logical_shift_right,
                )
                if dep is not None:
                    add_dep_helper(sh_i.ins, dep.ins, sync=True,
                                   reason="rsqrt waits TTR accum_out")
                magic = smallp.tile([128, 16], i32, tag="magic")
                nc.vector.memset(magic[:], 0x5F3759DF)
                y0i = smallp.tile([128, 16], i32, tag="y0i")
                nc.vector.tensor_sub(y0i[:], magic[:], ii[:])
                nsqh = smallp.tile([128, 16], f32, tag="nsqh")
                nc.vector.tensor_scalar_mul(nsqh[:], nsq[g][:], 0.5)
                y0 = y0i[:].bitcast(f32)
                t = smallp.tile([128, 16], f32, tag="t")
                for _ in range(2):
                    eng.tensor_mul(t[:], y0, y0)
                    eng.tensor_mul(t[:], t[:], nsqh[:])
                    eng.tensor_sub(t[:], c15[:], t[:])
                    eng.tensor_mul(rno[g][:], y0, t[:])
                    y0 = rno[g][:]

            def norm_chunk_act(g, q):
                """head variant: rno via ACT ln/exp (ACT is idle early)."""
                lnv = smallp.tile([128, 4], f32, tag="lnv")
                nc.scalar.activation(lnv[:], nsq[g][:, q * 4 : (q + 1) * 4],
                                     AF.Ln, bias=eps2[:, 0:1])
                nc.scalar.activation(rno[g][:, q * 4 : (q + 1) * 4],
                                     lnv[:], AF.Exp, scale=-0.5)

            def scale_chunk(g, q, dve=False):
                """zbg = xg * rno (broadcast TT per 512-col chunk, f32->bf16).

                GPSIMD in steady state; DVE for the latency-critical head.
                """
                eng = nc.vector if dve else nc.gpsimd
                sl = slice(q * 512, (q + 1) * 512)
                b = rno[g][:, 4 * q : 4 * q + 4].broadcast_to([128, 4, 128])
                eng.tensor_mul(
                    zbg[g][:, sl].rearrange("p (s d) -> p s d", s=4),
                    xg[g][:, sl].rearrange("p (s d) -> p s d", s=4),
                    b,
                )

            def transpose_chunk(g, q, ptr, dep=None, copy_dve=False):
                """PE-transpose 512 cols of zbg into psum, copy to zbT.

                ptr is a [128, 512] bf16 view carved out of a psum tile
                that has just been consumed, so no extra PSUM slot is held.
                """
                for j in range(4):
                    s = 4 * q + j
                    tr = nc.tensor.transpose(
                        ptr[:, j * 128 : (j + 1) * 128],
                        zbg[g][:, s * 128 : (s + 1) * 128],
                        ident[:],
                    )
                    if dep is not None and j == 0:
                        add_dep_helper(tr.ins, dep.ins, sync=True,
                                       reason="transpose WAR on psum consumer")
                nc.vector.tensor_copy(zbT[g][:, q * 512 : (q + 1) * 512], ptr[:])

            last_exp = [None]
            last_dve = [None]

            def extract_chunk(g, m, pt):
                """exp + row-sum of one [128, 2048] psum chunk."""
                col = g * RT + m
                if (g, m) in DVE_CHUNKS:
                    # Schraudolph: bf16_bits(exp(10*s)) = int16(s*A16 + B16)
                    k = m % 2
                    cons = nc.vector.tensor_scalar(
                        out=ei[k][:].bitcast(i16),
                        in0=pt[:],
                        scalar1=A16,
                        scalar2=B16,
                        op0=OP.mult,
                        op1=OP.add,
                    )
                    # 2x-mode reduce: copy-TS with accum_out on native bf16
                    red = nc.vector.tensor_scalar(
                        out=eb[:],
                        in0=ei[k][:],
                        scalar1=1.0,
                        scalar2=0.0,
                        op0=OP.mult,
                        op1=OP.add,
                        accum_out=racc[:, col : col + 1],
                    )
                    last_dve[0] = red
                else:
                    cons = nc.scalar.activation(
                        et[m % 2][:], pt[:], AF.Exp, scale=ESC,
                        accum_out=racc[:, col : col + 1],
                    )
                    last_exp[0] = cons
                return cons

            # ---- head: group 0 pipelined at 512-col granularity.  Slot A
            # hosts the m=0 sim strips; slot B's bf16 view hosts the
            # transpose scratch and a dummy-matmul warm strip.  Dummy
            # matmuls keep the PE HAM clock gate warm (cold PE runs
            # 1.2 GHz, warm 2.4 GHz).
            dumm = bigp.tile([128, 128], bf16, tag="dumm")
            nc.vector.memset(dumm[:], 0.0)
            ptA = psump.tile([128, GCOLS], f32, tag="pt", name="ptA")
            ptB = psump.tile([128, GCOLS], f32, tag="pt", name="ptB")

            def pe_warm(t, n):
                for _ in range(n):
                    nc.tensor.matmul(t[:, 1536:1664], ident[:], dumm[:])

            pe_warm(ptB, 40)
            load_group(0)
            trB = ptB.bitcast(bf16)
            for q in range(4):
                norm_chunk(0, q)
                norm_chunk_act(0, q)
                scale_chunk(0, q, dve=True)
                transpose_chunk(0, q, trB[:, q * 512 : (q + 1) * 512],
                                copy_dve=True)
                if q < 3:
                    pe_warm(ptB, 8)
                nc.tensor.matmul(
                    ptA[:, q * 512 : (q + 1) * 512],
                    zbT[0][:, 0:128],
                    zbT[0][:, q * 512 : (q + 1) * 512],
                )
                nc.scalar.activation(
                    et[0][:, q * 512 : (q + 1) * 512],
                    ptA[:, q * 512 : (q + 1) * 512],
                    AF.Exp, scale=ESC,
                    accum_out=racc[:, G * RT + q : G * RT + q + 1],
                )
            load_group(1)
            # ---- main stream: g outer, m inner, [128, 2048] chunks on two
            # ping-pong PSUM slots.  Group g+1's transposes run as bursts
            # of 8 into the just-consumed psum tile.
            slots = [ptA, ptB]
            si = [1]
            prev_cons = [None]
            for g in range(G):
                ms = list(range(1, RT)) if g == 0 else list(range(RT))
                for i, m in enumerate(ms):
                    pt = slots[si[0]]
                    si[0] ^= 1
                    # k descending: region [0:512] (transpose-scratch overlap)
                    # is written last, giving burst copies time to drain.
                    for k in (3, 2, 1, 0):
                        nc.tensor.matmul(
                            pt[:, k * 512 : (k + 1) * 512],
                            zbT[0][:, m * 128 : (m + 1) * 128],
                            zbT[g][:, k * 512 : (k + 1) * 512],
                        )
                    cons = extract_chunk(g, m, pt)
                    if g + 1 < G:
                        if i == 0:
                            for q in range(4):
                                norm_chunk(g + 1, q)
                        elif i == 1:
                            rsqrt_group(g + 1)
                        elif i in (2, 3):
                            scale_chunk(g + 1, 2 * (i - 2))
                            scale_chunk(g + 1, 2 * (i - 2) + 1)
                        elif i in (4, 6):
                            # borrow the *other* slot: its extract (i-1)
                            # finished while chunk i was being filled, so
                            # the transposes start without stalling PE.
                            trv = slots[si[0]].bitcast(bf16)
                            q0 = 0 if i == 4 else 2
                            transpose_chunk(g + 1, q0, trv[:, 0:512],
                                            dep=prev_cons[0])
                            transpose_chunk(g + 1, q0 + 1, trv[:, 512:1024],
                                            dep=prev_cons[0])
                        elif i == 5 and g + 2 < G:
                            load_group(g + 2)
                    prev_cons[0] = cons
                if g == 2:
                    # positives: pos[p, s] = sum_d zbg0[p,s,d] * zbg2[p,s,d]
                    pz = bigp.tile([128, RPC], bf16, tag="pz")
                    nc.gpsimd.tensor_mul(pz[:], zbg[0][:, 0:RPC], zbg[2][:, 0:RPC])
                    nc.vector.tensor_reduce(
                        pos[:],
                        pz[:].rearrange("p (s d) -> p s d", s=RT),
                        axis=AX.X, op=OP.add,
                    )

            # ---- epilogue: loss = ln(rowsum - e^10) - 10*pos ----
            tot = smallp.tile([128, RT], f32, tag="tot")
            t_i = nc.vector.tensor_reduce(
                tot[:],
                racc[:, 0 : G * RT].rearrange("p (g m) -> p m g", m=RT),
                axis=AX.X, op=OP.add,
            )
            if last_exp[0] is not None:
                add_dep_helper(t_i.ins, last_exp[0].ins, sync=True,
                               reason="epilogue waits last ACT accum_out")
            if last_dve[0] is not None:
                add_dep_helper(t_i.ins, last_dve[0].ins, sync=True,
                               reason="epilogue waits last DVE accum_out")
            th = smallp.tile([128, 1], f32, tag="th")
            nc.vector.tensor_reduce(
                th[:], racc[:, G * RT : G * RT + 4], axis=AX.X, op=OP.add
            )
            # fold the head sub-chunk sums into m=0
            nc.vector.tensor_add(tot[:, 0:1], tot[:, 0:1], th[:])
            ndall = smallp.tile([128, RT], f32, tag="ndall")
            nc.vector.tensor_scalar_add(ndall[:], tot[:], -E10)
            lnd = smallp.tile([128, RT], f32, tag="lnd")
            nc.scalar.activation(lnd[:], ndall[:], AF.Ln)
            lt = smallp.tile([128, RT], f32, tag="lt")
            nc.vector.scalar_tensor_tensor(
                out=lt[:], in0=pos[:], scalar=-ESC, in1=lnd[:],
                op0=OP.mult, op1=OP.add,
            )
            nc.sync.dma_start(out=out, in_=lt[:])

    nc.compile()
    return nc


def _get_nc():
    if "nc" not in _CACHE:
        _CACHE["nc"] = _build()
    return _CACHE["nc"]


def kernel(stacked_batch: np.ndarray) -> np.ndarray:
    global LAST_RESULTS
    from concourse.bass_utils import run_bass_kernel_spmd

    nc = _get_nc()
    xf = np.ascontiguousarray(np.asarray(stacked_batch, dtype=np.float32))
    assert xf.shape == (N, D)

    ident = np.eye(128, dtype=np.float32)
    in_maps = [
        {"x": np.ascontiguousarray(np.roll(xf, -c * RPC, axis=0)), "ident": ident}
        for c in range(NCORES)
    ]
    res = run_bass_kernel_spmd(
        nc,
        in_maps,
        core_ids=list(range(NCORES)),
        trace=bool(os.environ.get("BASS_TRACE")),
    )
    LAST_RESULTS = res
    total = 0.0
    for c in range(NCORES):
        total += float(np.asarray(res.results[c]["loss_parts"], dtype=np.float64).sum())
    return np.float32(total / N)


# revision 23
# speedup vs baseline: 1.2791x; 1.2020x over previous
"""Baseline NT-Xent kernel (98µs) — restored for device-health check."""

import os

import numpy as np

N = 8192
D = 128
NCORES = 8
RPC = N // NCORES          # rows per core = 1024
G = 4                      # column groups
GCOLS = N // G             # 2048 columns per group
RT = RPC // 128            # row tiles per core = 8
ESC = 10.0                 # 1 / temperature
E10 = float(np.exp(10.0))  # diagonal exp value (z normalized -> sim_ii = 10)

# int16 Schraudolph: bf16_bits(exp(10*s)) ~ int16(s*A16 + B16).  A 2-byte
# packed output runs the DVE tensor_scalar in 2x mode (measured).
A16 = 10.0 * (2.0 ** 7) / float(np.log(2.0))    # 1846.63
B16 = float(127 * 2 ** 7 - 7.25)                # calibrated for zero mean err

# Chunks (g, m) computed on the DVE instead of ScalarE. g=0 must stay on
# ACT (diagonal runs through exact exp so the e^10 subtraction cancels).
DVE_CHUNKS = {(g, m, h) for g in (1, 2, 3) for m in (2, 5) for h in (0, 1)}
DVE_CHUNKS -= {(1, 2, 0), (1, 2, 1)}

_CACHE = {}
LAST_RESULTS = None


def _patch_act_tables():
    """Force Exp/Ln onto the combined natural_log_exp_and_others table set."""
    if _CACHE.get("act_patched"):
        return
    import functools

    import concourse.bacc as bacc_mod
    import concourse.bass_interp as interp_mod
    import concourse.hw_specs as hw_specs
    import concourse.mybir as mybir

    AF = mybir.ActivationFunctionType
    orig = hw_specs.get_activation_tables

    @functools.cache
    def patched(arch):
        out = {}
        for name, funcs in orig(arch).items():
            if name != "natural_log_exp_and_others":
                funcs = funcs - {AF.Exp, AF.Ln}
            out[name] = funcs
        return out

    hw_specs.get_activation_tables = patched
    bacc_mod.get_activation_tables = patched
    interp_mod.get_activation_tables = patched
    _CACHE["act_patched"] = True


def _build():
    import concourse.mybir as mybir
    import concourse.tile as tile
    from concourse import bacc

    _patch_act_tables()

    f32 = mybir.dt.float32
    bf16 = mybir.dt.bfloat16
    i16 = mybir.dt.int16
    i32 = mybir.dt.int32
    AX = mybir.AxisListType
    OP = mybir.AluOpType
    AF = mybir.ActivationFunctionType

    nc = bacc.Bacc(
        "TRN2",
        target_bir_lowering=False,
        debug=False,
        enable_asserts=False,
        num_devices=NCORES,
    )
    x = nc.dram_tensor("x", [N, D], f32, kind="ExternalInput").ap()
    ident_in = nc.dram_tensor("ident", [128, 128], f32, kind="ExternalInput").ap()
    out = nc.dram_tensor("loss_parts", [128, RT], f32, kind="ExternalOutput").ap()

    with tile.TileContext(nc) as tc:
        with (
            tc.tile_pool(name="const", bufs=1) as constp,
            tc.tile_pool(name="big", bufs=1) as bigp,
            tc.tile_pool(name="small", bufs=2) as smallp,
            tc.tile_pool(name="psum", bufs=4, space="PSUM") as psump,
        ):
            ident = constp.tile([128, 128], bf16, tag="ident")
            identf = constp.tile([128, 128], f32, tag="identf")
            nc.scalar.dma_start(out=identf[:], in_=ident_in)
            nc.vector.tensor_copy(ident[:], identf[:])

            # Touch Ln+Exp so the ACT table load starts early.
            warm = constp.tile([128, 1], f32, tag="warm")
            nc.vector.memset(warm[:], 1.0)
            nc.scalar.activation(warm[:], warm[:], AF.Ln)
            nc.scalar.activation(warm[:], warm[:], AF.Exp)

            eps2 = constp.tile([128, 1], f32, tag="eps2")
            nc.vector.memset(eps2[:], 1e-16)

            xg = [bigp.tile([128, GCOLS], f32, tag=f"xg{g}", name=f"xg{g}") for g in range(G)]
            sq = [bigp.tile([128, GCOLS], f32, tag=f"sq{g}", name=f"sq{g}") for g in range(G)]
            zbg = [bigp.tile([128, GCOLS], bf16, tag=f"zbg{g}", name=f"zbg{g}") for g in range(G)]
            zbT = [bigp.tile([128, GCOLS], bf16, tag=f"zbT{g}", name=f"zbT{g}") for g in range(G)]
            nsq = [bigp.tile([128, 16], f32, tag=f"nsq{g}", name=f"nsq{g}") for g in range(G)]
            rno = [bigp.tile([128, 16], f32, tag=f"rno{g}", name=f"rno{g}") for g in range(G)]
            et = [bigp.tile([128, GCOLS], f32, tag=f"et{k}", name=f"et{k}") for k in range(2)]
            ei = [bigp.tile([128, GCOLS], i16, tag=f"ei{k}", name=f"ei{k}") for k in range(2)]

            racc = constp.tile([128, 2 * G * RT + 4], f32, tag="racc")   # ACT sums
            pos = constp.tile([128, RT], f32, tag="pos")

            def load_group(g, qs=range(4)):
                for q in qs:
                    src = x[g * GCOLS + q * 512 : g * GCOLS + (q + 1) * 512, :]
                    src = src.rearrange("(p s) d -> p s d", p=128)
                    dst = xg[g][:, q * 512 : (q + 1) * 512].rearrange(
                        "p (s d) -> p s d", s=4
                    )
                    eng = nc.sync if q % 2 == 0 else nc.scalar
                    eng.dma_start(out=dst, in_=src)

            def rsqrt_group(g):
                ii = smallp.tile([128, 16], i32, tag="ii")
                nc.vector.tensor_scalar(
                    out=ii[:], in0=nsq[g][:].bitcast(i32),
                    scalar1=1, scalar2=None, op0=OP.logical_shift_right,
                )
                magic = smallp.tile([128, 16], i32, tag="magic")
                nc.vector.memset(magic[:], 0x5F3759DF)
                y0i = smallp.tile([128, 16], i32, tag="y0i")
                nc.vector.tensor_sub(y0i[:], magic[:], ii[:])
                y0 = y0i[:].bitcast(f32)
                t = smallp.tile([128, 16], f32, tag="t")
                for _ in range(2):
                    nc.vector.tensor_mul(t[:], y0, y0)
                    nc.vector.scalar_tensor_tensor(
                        out=t[:], in0=t[:], scalar=-0.5, in1=nsq[g][:],
                        op0=OP.mult, op1=OP.mult,
                    )
                    nc.vector.tensor_scalar_add(t[:], t[:], 1.5)
                    nc.vector.tensor_mul(rno[g][:], y0, t[:])
                    y0 = rno[g][:]

            def norm_chunk(g, q, dve, rno_act=True):
                sl = slice(q * 512, (q + 1) * 512)
                eng = nc.vector if dve else nc.gpsimd
                eng.tensor_mul(sq[g][:, sl], xg[g][:, sl], xg[g][:, sl])
                nc.vector.tensor_reduce(
                    nsq[g][:, q * 4 : (q + 1) * 4],
                    sq[g][:, sl].rearrange("p (s d) -> p s d", s=4),
                    axis=AX.X, op=OP.add,
                )
                if not rno_act:
                    return
                lnv = smallp.tile([128, 4], f32, tag="lnv")
                nc.scalar.activation(lnv[:], nsq[g][:, q * 4 : (q + 1) * 4],
                                     AF.Ln, bias=eps2[:, 0:1])
                nc.scalar.activation(rno[g][:, q * 4 : (q + 1) * 4],
                                     lnv[:], AF.Exp, scale=-0.5)

            def scale_chunk(g, q, dve=True):
                eng = nc.vector if dve else nc.gpsimd
                for j in range(4):
                    s = 4 * q + j
                    eng.tensor_scalar_mul(
                        zbg[g][:, s * 128 : (s + 1) * 128],
                        xg[g][:, s * 128 : (s + 1) * 128],
                        rno[g][:, s : s + 1],
                    )

            def transpose_chunk(g, q, ptr, dep=None):
                from bass_rust import add_dep_helper
                for j in range(4):
                    s = 4 * q + j
                    tr = nc.tensor.transpose(
                        ptr[:, j * 128 : (j + 1) * 128],
                        zbg[g][:, s * 128 : (s + 1) * 128],
                        ident[:],
                    )
                    if dep is not None and j == 0:
                        add_dep_helper(tr.ins, dep.ins, sync=True,
                                       reason="transpose WAR on psum consumer")
                nc.vector.tensor_copy(
                    zbT[g][:, q * 512 : (q + 1) * 512], ptr[:]
                )

            last_exp = [None]

            def mm_chunk(g, m, h):
                pt = psump.tile([128, 1024], f32, tag="pt")
                lhs = zbT[0][:, m * 128 : (m + 1) * 128]
                for k in range(2):
                    c = h * 1024 + k * 512
                    nc.tensor.matmul(
                        pt[:, k * 512 : (k + 1) * 512],
                        lhs,
                        zbT[g][:, c : c + 512],
                    )
                col = (g * RT + m) * 2 + h
                if (g, m, h) in DVE_CHUNKS:
                    cons = nc.vector.tensor_scalar(
                        out=ei[h][:, 0:1024],
                        in0=pt[:],
                        scalar1=A16,
                        scalar2=B16,
                        op0=OP.mult,
                        op1=OP.add,
                    )
                    nc.vector.tensor_reduce(
                        racc[:, col : col + 1],
                        ei[h][:, 0:1024].bitcast(bf16),
                        axis=AX.X, op=OP.add,
                    )
                else:
                    cons = nc.scalar.activation(
                        et[h][:, 0:1024], pt[:], AF.Exp, scale=ESC,
                        accum_out=racc[:, col : col + 1],
                    )
                    last_exp[0] = cons
                return pt, cons

            dumm = bigp.tile([128, 128], bf16, tag="dumm")
            nc.vector.memset(dumm[:], 0.0)
            pth = [psump.tile([128, 1024], f32, tag="pt", name=f"pth{q}")
                   for q in range(4)]

            def pe_warm(t, n):
                for _ in range(n):
                    nc.tensor.matmul(t[:, 512:640], ident[:], dumm[:])

            pe_warm(pth[0], 40)
            load_group(0)
            from bass_rust import add_dep_helper as _adh2
            nrm_i = None
            for q in range(4):
                sl = slice(q * 512, (q + 1) * 512)
                nc.vector.tensor_mul(sq[0][:, sl], xg[0][:, sl], xg[0][:, sl])
                nrm_i = nc.vector.tensor_reduce(
                    nsq[0][:, q * 4 : (q + 1) * 4],
                    sq[0][:, sl].rearrange("p (s d) -> p s d", s=4),
                    axis=AX.X, op=OP.add,
                )
            lnv = smallp.tile([128, 16], f32, tag="lnv16")
            ln_i = nc.scalar.activation(lnv[:], nsq[0][:], AF.Ln,
                                        bias=eps2[:, 0:1])
            _adh2(ln_i.ins, nrm_i.ins, sync=True, reason="ln waits norms")
            nc.scalar.activation(rno[0][:], lnv[:], AF.Exp, scale=-0.5)
            for q in range(4):
                ph = pth[q]
                scale_chunk(0, q)
                trv = ph.bitcast(bf16)[:, 1536:2048]
                transpose_chunk(0, q, trv)
                if q < 3:
                    pe_warm(pth[q + 1], 8)
                nc.tensor.matmul(
                    ph[:, 0:512],
                    zbT[0][:, 0:128],
                    zbT[0][:, q * 512 : (q + 1) * 512],
                )
                nc.scalar.activation(
                    et[0][:, q * 512 : (q + 1) * 512],
                    ph[:, 0:512],
                    AF.Exp, scale=ESC,
                    accum_out=racc[:, 2 * G * RT + q : 2 * G * RT + q + 1],
                )
            load_group(1)
            for g in range(G):
                if g == 0:
                    mh_order = [(m, h) for m in range(1, RT) for h in range(2)]
                else:
                    mh_order = [(m, h) for m in range(RT) for h in range(2)]
                for i, (m, h) in enumerate(mh_order):
                    pt, cons = mm_chunk(g, m, h)
                    if g + 1 < G:
                        bpos = {3: 0, 9: 2} if g == 0 else {5: 0, 11: 2}
                        burst = bpos.get(i)
                        if burst is not None:
                            trv = pt.bitcast(bf16)
                            transpose_chunk(g + 1, burst, trv[:, 0:512], dep=cons)
                            transpose_chunk(g + 1, burst + 1, trv[:, 512:1024], dep=cons)
                    if g + 1 < G:
                        if i == 0:
                            for q in range(4):
                                norm_chunk(g + 1, q, dve=False, rno_act=False)
                            rsqrt_group(g + 1)
                        elif i == 2:
                            for q in range(4):
                                scale_chunk(g + 1, q)
                        elif i == 12 and g + 2 < G:
                            load_group(g + 2)
                if g == 2:
                    pz = bigp.tile([128, RPC], bf16, tag="pz")
                    nc.vector.tensor_mul(pz[:], zbg[0][:, 0:RPC], zbg[2][:, 0:RPC])
                    nc.vector.tensor_reduce(
                        pos[:],
                        pz[:].rearrange("p (s d) -> p s d", s=RT),
                        axis=AX.X, op=OP.add,
                    )

            from bass_rust import add_dep_helper as _adh
            tot32 = smallp.tile([128, G * RT], f32, tag="tot32")
            t32_i = nc.vector.tensor_reduce(
                tot32[:],
                racc[:, 0 : 2 * G * RT].rearrange("p (gm h) -> p gm h", h=2),
                axis=AX.X, op=OP.add,
            )
            if last_exp[0] is not None:
                _adh(t32_i.ins, last_exp[0].ins, sync=True,
                     reason="epilogue waits last ACT accum_out")
            tot = smallp.tile([128, RT], f32, tag="tot")
            nc.vector.tensor_reduce(
                tot[:],
                tot32[:].rearrange("p (g m) -> p m g", m=RT),
                axis=AX.X, op=OP.add,
            )
            th = smallp.tile([128, 1], f32, tag="th")
            nc.vector.tensor_reduce(
                th[:], racc[:, 2 * G * RT : 2 * G * RT + 4], axis=AX.X, op=OP.add
            )
            nc.vector.tensor_add(tot[:, 0:1], tot[:, 0:1], th[:])
            ndall = smallp.tile([128, RT], f32, tag="ndall")
            nc.vector.tensor_scalar_add(ndall[:], tot[:], -E10)
            lnd = smallp.tile([128, RT], f32, tag="lnd")
            nc.scalar.activation(lnd[:], ndall[:], AF.Ln)
            lt = smallp.tile([128, RT], f32, tag="lt")
            nc.vector.scalar_tensor_tensor(
                out=lt[:], in0=pos[:], scalar=-ESC, in1=lnd[:],
                op0=OP.mult, op1=OP.add,
            )
            nc.sync.dma_start(out=out, in_=lt[:])

    nc.compile()
    return nc


def _get_nc():
    if "nc" not in _CACHE:
        _CACHE["nc"] = _build()
    return _CACHE["nc"]


def kernel(stacked_batch: np.ndarray) -> np.ndarray:
    global LAST_RESULTS
    from concourse.bass_utils import run_bass_kernel_spmd

    nc = _get_nc()
    xf = np.ascontiguousarray(np.asarray(stacked_batch, dtype=np.float32))
    assert xf.shape == (N, D)

    ident = np.eye(128, dtype=np.float32)
    in_maps = [
        {"x": np.ascontiguousarray(np.roll(xf, -c * RPC, axis=0)), "ident": ident}
        for c in range(NCORES)
    ]
    res = run_bass_kernel_spmd(
        nc,
        in_maps,
        core_ids=list(range(NCORES)),
        trace=bool(os.environ.get("BASS_TRACE")),
    )
    LAST_RESULTS = res
    total = 0.0
    for c in range(NCORES):
        total += float(np.asarray(res.results[c]["loss_parts"], dtype=np.float64).sum())
    return np.float32(total / N)
